# revision 1
# baseline (speedup 1.0000x reference)
import os
import sys

import numpy as np

if "/opt/trn_rl_repo" not in sys.path:
    sys.path.insert(0, "/opt/trn_rl_repo")

import concourse.bass as bass
import concourse.mybir as mybir
import concourse.tile as tile
from concourse import bacc
from concourse.bass_utils import run_bass_kernel_spmd

P = 128
B, N, E = 64, 10000, 320000
LAMBDA_PHY = 0.3
NCORES = 8
EPC = E // NCORES            # 40000 real edges per core
G = (EPC + P - 1) // P       # 313 slot groups per core
EPAD = G * P                 # 40064 (64 pad entries per core)
CHUNK_SLOTS = 8              # 1024 idxs/gather: HW limit is <=1024 per dma_gather
NCHUNKS = (G + CHUNK_SLOTS - 1) // CHUNK_SLOTS  # 39 full chunks + 1-slot tail
PAIR_SLOTS = 2 * CHUNK_SLOTS # compute batches two gather chunks per vector op
NPAIRS = NCHUNKS // 2        # 20
NDL = N // NCORES            # 1250 data-loss columns per core
DL_F = B * NDL // P          # 625 free-dim elems for [128, 625] reshape

FP = mybir.dt.float32
I16 = mybir.dt.int16

LAST_EXEC_NS = None
LAST_PROFILE = None

_NC_CACHE = {}


def _build_nc():
    if "nc" in _NC_CACHE:
        return _NC_CACHE["nc"]
    nc = bacc.Bacc(None, target_bir_lowering=False, num_swdge_queues=4)

    table_d = nc.declare_dram_parameter("table", [N, P], FP, isOutput=False)
    sidx_d = nc.declare_dram_parameter("sidx", [P, EPAD // 16], I16, isOutput=False)
    didx_d = nc.declare_dram_parameter("didx", [P, EPAD // 16], I16, isOutput=False)
    c0_d = nc.declare_dram_parameter("c0a", [P, G], FP, isOutput=False)
    c1_d = nc.declare_dram_parameter("c1a", [P, G], FP, isOutput=False)
    c2_d = nc.declare_dram_parameter("c2a", [P, G], FP, isOutput=False)
    pdl_d = nc.declare_dram_parameter("pdl", [P, DL_F], FP, isOutput=False)
    tdl_d = nc.declare_dram_parameter("tdl", [P, DL_F], FP, isOutput=False)
    out_d = nc.declare_dram_parameter("partials", [P, 2], FP, isOutput=True)

    with tile.TileContext(nc) as tc:
        with tc.tile_pool(name="sbuf", bufs=1) as pool:
            sidx_t = pool.tile([P, EPAD // 16], I16)
            didx_t = pool.tile([P, EPAD // 16], I16)
            c0_t = pool.tile([P, G], FP)
            c1_t = pool.tile([P, G], FP)
            c2_t = pool.tile([P, G], FP)
            pdl_t = pool.tile([P, DL_F], FP)
            tdl_t = pool.tile([P, DL_F], FP)
            dd_t = pool.tile([P, DL_F], FP)
            sq_dl = pool.tile([P, DL_F], FP)
            phy_acc = pool.tile([P, 1], FP)
            dacc = pool.tile([P, 1], FP)
            chunk_accs = pool.tile([P, NPAIRS], FP)

            NBUF = 3
            gs_t = [
                pool.tile([P, PAIR_SLOTS, P], FP, name=f"gs{i}") for i in range(NBUF)
            ]
            gd_t = [
                pool.tile([P, PAIR_SLOTS, P], FP, name=f"gd{i}") for i in range(NBUF)
            ]
            a0_t = pool.tile([P, PAIR_SLOTS, B], FP)
            a1_t = pool.tile([P, PAIR_SLOTS, B], FP)
            b_t = pool.tile([P, PAIR_SLOTS, B], FP)
            c_t = pool.tile([P, PAIR_SLOTS, B], FP)
            r_t = pool.tile([P, PAIR_SLOTS, B], FP)

            nc.sync.dma_start(out=sidx_t[:], in_=sidx_d[:])
            nc.sync.dma_start(out=didx_t[:], in_=didx_d[:])
            nc.sync.dma_start(out=c0_t[:], in_=c0_d[:])
            nc.sync.dma_start(out=c1_t[:], in_=c1_d[:])
            nc.sync.dma_start(out=c2_t[:], in_=c2_d[:])
            nc.sync.dma_start(out=pdl_t[:], in_=pdl_d[:])
            nc.sync.dma_start(out=tdl_t[:], in_=tdl_d[:])

            # data loss partial: sum((pred - target)^2) over this core's shard
            # (tensor_tensor_reduce crashes the device on this toolchain, so
            # square + separate tensor_reduce instead)
            nc.vector.tensor_tensor(
                out=dd_t[:], in0=pdl_t[:], in1=tdl_t[:], op=mybir.AluOpType.subtract
            )
            nc.vector.tensor_tensor(
                out=sq_dl[:], in0=dd_t[:], in1=dd_t[:], op=mybir.AluOpType.mult
            )
            nc.vector.tensor_reduce(
                out=dacc[:],
                in_=sq_dl[:],
                axis=mybir.AxisListType.X,
                op=mybir.AluOpType.add,
            )

            for j in range(NPAIRS):
                gs = gs_t[j % NBUF]
                gd = gd_t[j % NBUF]

                for h in range(2):
                    k = 2 * j + h
                    so = k * CHUNK_SLOTS
                    S = min(CHUNK_SLOTS, G - so)
                    sl0 = h * CHUNK_SLOTS

                    n_idx = S * P
                    n_real = min(EPC - so * P, n_idx)
                    col0 = so * 8  # slot*128/16
                    ncol = n_idx // 16

                    if n_real < n_idx:
                        # pad entries (negative idxs) are skipped by the
                        # gather; zero their slots so the residual comes out 0
                        pad_p0 = n_real - (S - 1) * P
                        sl = sl0 + S - 1
                        nc.vector.memset(gs[pad_p0:P, sl : sl + 1, :], 0.0)
                        nc.vector.memset(gd[pad_p0:P, sl : sl + 1, :], 0.0)

                    nc.gpsimd.dma_gather(
                        out_ap=gs[:, sl0 : sl0 + S, :],
                        in_ap=table_d[:, :],
                        idxs_ap=sidx_t[:, col0 : col0 + ncol],
                        num_idxs=n_idx,
                        num_idxs_reg=n_real,
                        elem_size=P,
                        queue_num=(2 * k) % 4,
                    )
                    nc.gpsimd.dma_gather(
                        out_ap=gd[:, sl0 : sl0 + S, :],
                        in_ap=table_d[:, :],
                        idxs_ap=didx_t[:, col0 : col0 + ncol],
                        num_idxs=n_idx,
                        num_idxs_reg=n_real,
                        elem_size=P,
                        queue_num=(2 * k + 1) % 4,
                    )

                po = j * PAIR_SLOTS
                S = min(PAIR_SLOTS, G - po)
                c0b = c0_t[:, po : po + S, None].to_broadcast([P, S, B])
                c1b = c1_t[:, po : po + S, None].to_broadcast([P, S, B])
                c2b = c2_t[:, po : po + S, None].to_broadcast([P, S, B])

                mul = mybir.AluOpType.mult
                sub = mybir.AluOpType.subtract
                # a0 = c0 * pred[src], a1 = c1 * prev[src], b = c2 * prev[dst]
                nc.vector.tensor_tensor(
                    out=a0_t[:, 0:S, :], in0=gs[:, 0:S, 0:B], in1=c0b, op=mul
                )
                nc.vector.tensor_tensor(
                    out=a1_t[:, 0:S, :], in0=gs[:, 0:S, B:P], in1=c1b, op=mul
                )
                nc.vector.tensor_tensor(
                    out=b_t[:, 0:S, :], in0=gd[:, 0:S, B:P], in1=c2b, op=mul
                )
                # r = pred[dst] - b - a0 - a1
                nc.vector.tensor_tensor(
                    out=c_t[:, 0:S, :], in0=gd[:, 0:S, 0:B], in1=b_t[:, 0:S, :], op=sub
                )
                nc.vector.tensor_tensor(
                    out=c_t[:, 0:S, :], in0=c_t[:, 0:S, :], in1=a0_t[:, 0:S, :], op=sub
                )
                nc.vector.tensor_tensor(
                    out=r_t[:, 0:S, :], in0=c_t[:, 0:S, :], in1=a1_t[:, 0:S, :], op=sub
                )
                # chunk_accs[:, j] = sum over (S, B) of r^2 per partition
                nc.vector.tensor_tensor(
                    out=b_t[:, 0:S, :], in0=r_t[:, 0:S, :], in1=r_t[:, 0:S, :], op=mul
                )
                nc.vector.tensor_reduce(
                    out=chunk_accs[:, j : j + 1],
                    in_=b_t[:, 0:S, :],
                    axis=mybir.AxisListType.XY,
                    op=mybir.AluOpType.add,
                )

            nc.vector.tensor_reduce(
                out=phy_acc[:],
                in_=chunk_accs[:],
                axis=mybir.AxisListType.X,
                op=mybir.AluOpType.add,
            )
            nc.sync.dma_start(out=out_d[:, 0:1], in_=phy_acc[:])
            nc.sync.dma_start(out=out_d[:, 1:2], in_=dacc[:])

    # Bacc.finalize runs the full lowering pipeline: wait splitting,
    # library loads for DMAGatherAnt, codegen_inst_isa_subclasses
    nc.finalize()
    _NC_CACHE["nc"] = nc
    return nc


def _wrap_idx(idx_pad: np.ndarray) -> np.ndarray:
    # dma_gather layout: index i lives at partition i%16, column i//16,
    # replicated across the 8 groups of 16 partitions
    w16 = idx_pad.reshape(EPAD // 16, 16).T  # [16, EPAD//16]
    return np.ascontiguousarray(np.tile(w16, (8, 1)))  # [128, EPAD//16]


def _arrange_coeff(c_shard: np.ndarray) -> np.ndarray:
    cp = np.zeros(EPAD, np.float32)
    cp[:EPC] = c_shard
    return np.ascontiguousarray(cp.reshape(G, P).T)  # [128, G]


def kernel(**inputs) -> np.ndarray:
    global LAST_EXEC_NS, LAST_PROFILE
    pred = np.ascontiguousarray(np.asarray(inputs["pred"], dtype=np.float32))
    target = np.ascontiguousarray(np.asarray(inputs["target"], dtype=np.float32))
    prev_target = np.ascontiguousarray(
        np.asarray(inputs["prev_target"], dtype=np.float32)
    )
    c0 = np.asarray(inputs["c0"], dtype=np.float32)
    c1 = np.asarray(inputs["c1"], dtype=np.float32)
    c2 = np.asarray(inputs["c2"], dtype=np.float32)
    edge_index = np.asarray(inputs["edge_index"])
    src = edge_index[0].astype(np.int16)
    dst = edge_index[1].astype(np.int16)

    # gather table: row n = [pred[:, n] | prev_target[:, n]]  (512B rows)
    table = np.ascontiguousarray(
        np.concatenate([pred.T, prev_target.T], axis=1), dtype=np.float32
    )

    in_maps = []
    for c in range(NCORES):
        esl = slice(c * EPC, (c + 1) * EPC)
        s_pad = np.full(EPAD, -1, np.int16)
        s_pad[:EPC] = src[esl]
        d_pad = np.full(EPAD, -1, np.int16)
        d_pad[:EPC] = dst[esl]
        nsl = slice(c * NDL, (c + 1) * NDL)
        in_maps.append(
            {
                "table": table,
                "sidx": _wrap_idx(s_pad),
                "didx": _wrap_idx(d_pad),
                "c0a": _arrange_coeff(c0[esl]),
                "c1a": _arrange_coeff(c1[esl]),
                "c2a": _arrange_coeff(c2[esl]),
                "pdl": np.ascontiguousarray(pred[:, nsl].reshape(P, DL_F)),
                "tdl": np.ascontiguousarray(target[:, nsl].reshape(P, DL_F)),
            }
        )

    nc = _build_nc()
    res = run_bass_kernel_spmd(nc, in_maps, list(range(NCORES)))
    LAST_EXEC_NS = res.exec_time_ns
    LAST_PROFILE = res.profile_json

    phy_sum = 0.0
    data_sum = 0.0
    for c in range(NCORES):
        part = np.asarray(res.results[c]["partials"], dtype=np.float64)
        phy_sum += part[:, 0].sum()
        data_sum += part[:, 1].sum()

    data_loss = data_sum / (B * N)
    phy_loss = phy_sum / (B * E)
    total = data_loss + LAMBDA_PHY * phy_loss
    return np.array([total, data_loss, phy_loss], dtype=np.float32)


if __name__ == "__main__":
    rng = np.random.default_rng(0)
    ins = {
        "pred": rng.standard_normal((B, N), dtype=np.float32),
        "target": rng.standard_normal((B, N), dtype=np.float32),
        "prev_target": rng.standard_normal((B, N), dtype=np.float32),
        "c0": rng.random(E, dtype=np.float32),
        "c1": rng.random(E, dtype=np.float32),
        "c2": rng.random(E, dtype=np.float32),
        "edge_index": rng.integers(0, N, (2, E)).astype(np.int64),
    }
    out = kernel(**ins)
    print("kernel out:", out)



# revision 5
# speedup vs baseline: 1.0511x; 1.0511x over previous
import os
import sys

import numpy as np
import ml_dtypes

if "/opt/trn_rl_repo" not in sys.path:
    sys.path.insert(0, "/opt/trn_rl_repo")

import concourse.bass as bass
import concourse.mybir as mybir
import concourse.tile as tile
from concourse import bacc
from concourse.bass_utils import run_bass_kernel_spmd

P = 128
B, N, E = 64, 10000, 320000
LAMBDA_PHY = 0.3
NCORES = 8
EPC = E // NCORES              # 40000 real edges per core

# quad/chunk geometry (per core)
QC = 1024                      # quads per compute chunk (8 qslots x 128 parts)
NCHUNK = 11                    # compute chunks; QPAD*4 >= worst-case padded edges
QPAD = NCHUNK * QC             # 11264 quads
EPADC = QPAD * 4               # 45056 edge slots
ESLOTS = EPADC // P            # 352 edge slots ([P, 352] coefficient layout)

# >1024 idxs per dma_gather call crashes the device (ucode cap)
SRC_CALL = int(os.environ.get("K_SRC_CALL", "1024"))   # idxs per src dma_gather
DST_CALL = int(os.environ.get("K_DST_CALL", "1024"))   # idxs per dst dma_gather
SCRATCH = int(os.environ.get("K_SCRATCH", "65536"))

NDL = N // NCORES              # 1250 data-loss columns per core
DL_F = B * NDL // P            # 625

FP = mybir.dt.float32
BF = mybir.dt.bfloat16
I16 = mybir.dt.int16

LAST_EXEC_NS = None
LAST_PROFILE = None

_NC_CACHE = {}


def _build_nc():
    if "nc" in _NC_CACHE:
        return _NC_CACHE["nc"]
    nc = bacc.Bacc(
        None,
        target_bir_lowering=False,
        num_swdge_queues=4,
        dynamic_dma_scratch_size=SCRATCH,
    )

    table_d = nc.declare_dram_parameter("table", [N, P], BF, isOutput=False)
    sidx_d = nc.declare_dram_parameter("sidx", [P, EPADC // 16], I16, isOutput=False)
    didx_d = nc.declare_dram_parameter("didx", [P, QPAD // 16], I16, isOutput=False)
    c0_d = nc.declare_dram_parameter("c0a", [P, ESLOTS], BF, isOutput=False)
    c1_d = nc.declare_dram_parameter("c1a", [P, ESLOTS], BF, isOutput=False)
    c2_d = nc.declare_dram_parameter("c2a", [P, ESLOTS], BF, isOutput=False)
    pdl_d = nc.declare_dram_parameter("pdl", [P, DL_F], FP, isOutput=False)
    tdl_d = nc.declare_dram_parameter("tdl", [P, DL_F], FP, isOutput=False)
    out_d = nc.declare_dram_parameter("partials", [P, 2], FP, isOutput=True)

    with tile.TileContext(nc) as tc:
        with tc.tile_pool(name="sbuf", bufs=1) as pool:
            sidx_t = pool.tile([P, EPADC // 16], I16)
            didx_t = pool.tile([P, QPAD // 16], I16)
            c0_t = pool.tile([P, ESLOTS], BF)
            c1_t = pool.tile([P, ESLOTS], BF)
            c2_t = pool.tile([P, ESLOTS], BF)
            pdl_t = pool.tile([P, DL_F], FP)
            tdl_t = pool.tile([P, DL_F], FP)
            dd_t = pool.tile([P, DL_F], FP)
            dsq_t = pool.tile([P, DL_F], FP)
            dacc = pool.tile([P, 1], FP)
            phy_acc = pool.tile([P, 1], FP)
            chunk_accs = pool.tile([P, NCHUNK], FP)

            NBUF = 3
            gs_t = [pool.tile([P, 32, P], BF, name=f"gs{i}") for i in range(NBUF)]
            qd_t = [pool.tile([P, 8, P], BF, name=f"qd{i}") for i in range(NBUF)]
            m0_t = [pool.tile([P, 32, B], BF, name=f"m0_{i}") for i in range(2)]
            m1_t = [pool.tile([P, 32, B], BF, name=f"m1_{i}") for i in range(2)]
            u_t = [pool.tile([P, 32, B], BF, name=f"u{i}") for i in range(2)]
            m2_t = [pool.tile([P, 8, B], BF, name=f"m2_{i}") for i in range(2)]
            r_t = [pool.tile([P, 32, B], BF, name=f"r{i}") for i in range(2)]
            sq_t = pool.tile([P, 32, B], BF)

            nc.sync.dma_start(out=sidx_t[:], in_=sidx_d[:])
            nc.sync.dma_start(out=didx_t[:], in_=didx_d[:])
            nc.sync.dma_start(out=c0_t[:], in_=c0_d[:])
            nc.sync.dma_start(out=c1_t[:], in_=c1_d[:])
            nc.sync.dma_start(out=c2_t[:], in_=c2_d[:])
            nc.sync.dma_start(out=pdl_t[:], in_=pdl_d[:])
            nc.sync.dma_start(out=tdl_t[:], in_=tdl_d[:])

            # data loss partial: sum((pred - target)^2), square+reduce on Act
            nc.vector.tensor_tensor(
                out=dd_t[:], in0=pdl_t[:], in1=tdl_t[:], op=mybir.AluOpType.subtract
            )
            nc.scalar.activation(
                out=dsq_t[:],
                in_=dd_t[:],
                func=mybir.ActivationFunctionType.Square,
                accum_out=dacc[:],
            )

            mul = mybir.AluOpType.mult
            sub = mybir.AluOpType.subtract
            add = mybir.AluOpType.add

            qn = [0]  # round-robin queue counter for gathers
            for k in range(NCHUNK):
                gs = gs_t[k % NBUF]
                qd = qd_t[k % NBUF]
                m0 = m0_t[k % 2]
                m1 = m1_t[k % 2]
                u = u_t[k % 2]
                m2 = m2_t[k % 2]
                r = r_t[k % 2]

                # src gather: 4096 idxs for this chunk (split into calls)
                base = k * 4096
                for ci in range(0, 4096, SRC_CALL):
                    i0 = base + ci
                    nc.gpsimd.dma_gather(
                        out_ap=gs[:, ci // 128 : (ci + SRC_CALL) // 128, :],
                        in_ap=table_d[:, :],
                        idxs_ap=sidx_t[:, i0 // 16 : (i0 + SRC_CALL) // 16],
                        num_idxs=SRC_CALL,
                        num_idxs_reg=SRC_CALL,
                        elem_size=P,
                        queue_num=qn[0] % 4,
                    )
                    qn[0] += 1
                # dst gather: 1024 quad idxs
                qbase = k * QC
                for ci in range(0, QC, DST_CALL):
                    i0 = qbase + ci
                    nc.gpsimd.dma_gather(
                        out_ap=qd[:, ci // 128 : (ci + DST_CALL) // 128, :],
                        in_ap=table_d[:, :],
                        idxs_ap=didx_t[:, i0 // 16 : (i0 + DST_CALL) // 16],
                        num_idxs=DST_CALL,
                        num_idxs_reg=DST_CALL,
                        elem_size=P,
                        queue_num=qn[0] % 4,
                    )
                    qn[0] += 1

                so = k * 32
                c0b = c0_t[:, so : so + 32, None].to_broadcast([P, 32, B])
                c1b = c1_t[:, so : so + 32, None].to_broadcast([P, 32, B])
                # u = c0*ps + c1*prs
                nc.vector.tensor_tensor(
                    out=m0[:], in0=gs[:, :, 0:B], in1=c0b, op=mul
                )
                nc.vector.tensor_tensor(
                    out=m1[:], in0=gs[:, :, B:P], in1=c1b, op=mul
                )
                nc.vector.tensor_tensor(out=u[:], in0=m0[:], in1=m1[:], op=add)
                # per j: m2 = c2*prd ; r_j = (pd - m2) - u_j
                for j in range(4):
                    sl = slice(8 * j, 8 * j + 8)
                    c2bj = c2_t[:, so + 8 * j : so + 8 * j + 8, None].to_broadcast(
                        [P, 8, B]
                    )
                    nc.vector.tensor_tensor(
                        out=m2[:], in0=qd[:, :, B:P], in1=c2bj, op=mul
                    )
                    nc.vector.tensor_tensor(
                        out=r[:, sl, :], in0=qd[:, :, 0:B], in1=m2[:], op=sub
                    )
                    nc.vector.tensor_tensor(
                        out=r[:, sl, :], in0=r[:, sl, :], in1=u[:, sl, :], op=sub
                    )
                # Act: square + accumulate -> chunk_accs[:, k]
                nc.scalar.activation(
                    out=sq_t[:],
                    in_=r[:],
                    func=mybir.ActivationFunctionType.Square,
                    accum_out=chunk_accs[:, k : k + 1],
                )

            nc.vector.tensor_reduce(
                out=phy_acc[:],
                in_=chunk_accs[:],
                axis=mybir.AxisListType.X,
                op=mybir.AluOpType.add,
            )
            nc.sync.dma_start(out=out_d[:, 0:1], in_=phy_acc[:])
            nc.sync.dma_start(out=out_d[:, 1:2], in_=dacc[:])

    nc.finalize()
    _NC_CACHE["nc"] = nc
    return nc


def _wrap_idx(idx: np.ndarray) -> np.ndarray:
    # dma_gather layout: index i lives at partition i%16, column i//16,
    # replicated across the 8 groups of 16 partitions
    n = idx.shape[0]
    w16 = idx.reshape(n // 16, 16).T
    return np.ascontiguousarray(np.tile(w16, (8, 1)))


def _prep_core(s, d, c0, c1, c2):
    """Build one core's padded quad-major edge arrays.

    Edges arrive sorted by dst. Each dst run is padded to a multiple of 4
    with synthetic edges (src=dst, c0=1, c1=c2=0) whose residual is exactly
    zero. Leftover quad slots are filled with node-0 synthetic edges.
    Returns (src_seq, dst_quad, c0_seq, c1_seq, c2_seq) where the _seq
    arrays are in gather-position order (length EPADC) and dst_quad has
    one entry per quad (length QPAD).
    """
    uds, counts = np.unique(d, return_counts=True)
    pad_counts = (-counts) % 4
    padded = counts + pad_counts
    tot = int(padded.sum())
    assert tot <= EPADC, f"padded edges {tot} > {EPADC}"

    starts = np.concatenate(([0], np.cumsum(padded)))[:-1]
    run_starts = np.concatenate(([0], np.cumsum(counts)))[:-1]
    pos = np.repeat(starts, counts) + (np.arange(len(d)) - np.repeat(run_starts, counts))

    dst_p = np.zeros(EPADC, np.int64)
    dst_p[:tot] = np.repeat(uds, padded)
    src_p = dst_p.copy()              # synthetic edges: src = dst
    c0_p = np.ones(EPADC, np.float32)  # synthetic: c0=1 -> r = pd - pd = 0
    c1_p = np.zeros(EPADC, np.float32)
    c2_p = np.zeros(EPADC, np.float32)
    src_p[pos] = s
    c0_p[pos] = c0
    c1_p[pos] = c1
    c2_p[pos] = c2

    # quad dst index (one per quad; all 4 edges of a quad share dst)
    dst_quad = dst_p[0::4]

    # edge (q, j) -> gather position (k*32 + j*8 + qq)*128 + p
    e = np.arange(EPADC)
    q, j = e >> 2, e & 3
    k = q // QC
    p = q % P
    qq = (q % QC) // P
    gpos = (k * 32 + j * 8 + qq) * P + p

    src_seq = np.empty(EPADC, np.int16)
    src_seq[gpos] = src_p.astype(np.int16)
    c0_seq = np.empty(EPADC, np.float32)
    c0_seq[gpos] = c0_p
    c1_seq = np.empty(EPADC, np.float32)
    c1_seq[gpos] = c1_p
    c2_seq = np.empty(EPADC, np.float32)
    c2_seq[gpos] = c2_p
    return src_seq, dst_quad.astype(np.int16), c0_seq, c1_seq, c2_seq


def _coeff_tile(seq: np.ndarray) -> np.ndarray:
    # gather-position order -> [P, ESLOTS] (pos = col*128 + p)
    return np.ascontiguousarray(
        seq.reshape(ESLOTS, P).T.astype(ml_dtypes.bfloat16)
    )


def kernel(**inputs) -> np.ndarray:
    global LAST_EXEC_NS, LAST_PROFILE
    pred = np.ascontiguousarray(np.asarray(inputs["pred"], dtype=np.float32))
    target = np.ascontiguousarray(np.asarray(inputs["target"], dtype=np.float32))
    prev_target = np.ascontiguousarray(
        np.asarray(inputs["prev_target"], dtype=np.float32)
    )
    c0 = np.asarray(inputs["c0"], dtype=np.float32)
    c1 = np.asarray(inputs["c1"], dtype=np.float32)
    c2 = np.asarray(inputs["c2"], dtype=np.float32)
    edge_index = np.asarray(inputs["edge_index"])
    src = edge_index[0].astype(np.int64)
    dst = edge_index[1].astype(np.int64)

    # sort edges by (dst, src); contiguous 1/8 chunks per core
    order = np.lexsort((src, dst))
    src_s, dst_s = src[order], dst[order]
    c0_s, c1_s, c2_s = c0[order], c1[order], c2[order]

    # gather table: row n = [pred[:, n] | prev_target[:, n]] in bf16 (256B)
    table = np.ascontiguousarray(
        np.concatenate([pred.T, prev_target.T], axis=1).astype(ml_dtypes.bfloat16)
    )

    in_maps = []
    for c in range(NCORES):
        esl = slice(c * EPC, (c + 1) * EPC)
        src_seq, dst_quad, c0_seq, c1_seq, c2_seq = _prep_core(
            src_s[esl], dst_s[esl], c0_s[esl], c1_s[esl], c2_s[esl]
        )
        nsl = slice(c * NDL, (c + 1) * NDL)
        in_maps.append(
            {
                "table": table,
                "sidx": _wrap_idx(src_seq),
                "didx": _wrap_idx(dst_quad),
                "c0a": _coeff_tile(c0_seq),
                "c1a": _coeff_tile(c1_seq),
                "c2a": _coeff_tile(c2_seq),
                "pdl": np.ascontiguousarray(pred[:, nsl].reshape(P, DL_F)),
                "tdl": np.ascontiguousarray(target[:, nsl].reshape(P, DL_F)),
            }
        )

    nc = _build_nc()
    res = run_bass_kernel_spmd(nc, in_maps, list(range(NCORES)))
    LAST_EXEC_NS = res.exec_time_ns
    LAST_PROFILE = res.profile_json

    phy_sum = 0.0
    data_sum = 0.0
    for c in range(NCORES):
        part = np.asarray(res.results[c]["partials"], dtype=np.float64)
        phy_sum += part[:, 0].sum()
        data_sum += part[:, 1].sum()

    data_loss = data_sum / (B * N)
    phy_loss = phy_sum / (B * E)
    total = data_loss + LAMBDA_PHY * phy_loss
    return np.array([total, data_loss, phy_loss], dtype=np.float32)


if __name__ == "__main__":
    rng = np.random.default_rng(0)
    ins = {
        "pred": rng.standard_normal((B, N), dtype=np.float32),
        "target": rng.standard_normal((B, N), dtype=np.float32),
        "prev_target": rng.standard_normal((B, N), dtype=np.float32),
        "c0": rng.random(E, dtype=np.float32),
        "c1": rng.random(E, dtype=np.float32),
        "c2": rng.random(E, dtype=np.float32),
        "edge_index": rng.integers(0, N, (2, E)).astype(np.int64),
    }
    out = kernel(**ins)
    # numpy check
    p64 = ins["pred"].astype(np.float64)
    t64 = ins["target"].astype(np.float64)
    pv64 = ins["prev_target"].astype(np.float64)
    s, d = ins["edge_index"]
    dl = np.mean((p64 - t64) ** 2)
    exp = (ins["c0"] * p64[:, s] + ins["c1"] * pv64[:, s] + ins["c2"] * pv64[:, d])
    res_ = p64[:, d] - exp
    pl = np.mean(res_ ** 2)
    ref = np.array([dl + LAMBDA_PHY * pl, dl, pl])
    print("kernel out:", out)
    print("numpy ref :", ref)
    print("rel err   :", np.abs(out - ref) / np.abs(ref))
